# revision 41
# baseline (speedup 1.0000x reference)
"""GAT-style dense-mask attention (gnn_message_passing) on 8 trn2 cores.

Sharding: core c owns head h=c//2 and query rows [r0, r0+2048), r0=(c%2)*2048.
Inputs are pre-sliced/rolled on host so all 8 cores run one identical SPMD
program; outputs are [2048,128] blocks reassembled on host.

Math (per core, node order rolled so own rows come first):
  support = X @ Wh                      [4096, 128]
  f1 = X @ (Wh @ u), f2 = X @ (Wh @ v)  [4096]
  p[j,i] = adj[i,j] * exp(prelu_0.2(f1[j]+f2[i]))     (j on partitions)
  out[i,:] = (p.T @ [support|1])[:, :128] / (p.T @ [support|1])[:, 128]
             + X[i] @ proj_w_h + bias_h + proj_b_h
The mask multiply uses exp underflow (adj=0 -> p=0) instead of the -1e30
trick, which is exactly equivalent because softmax is shift-invariant.

Schedule: stage-2 pools are allocated before stage-1's streaming input pool
so their SBUF ranges are disjoint — attention tiles (ACT prelu/exp, DVE/GPS
mask) overlap the tail of the support pass on PE.
"""

import os

import ml_dtypes
import numpy as np

N = 4096
IN = 512
D = 128
H = 4
NCORES = 8
RPC = N // 2          # query rows per core
JCH = N // 128        # 32 source-node chunks
ICH = RPC // 128      # 16 query-row chunks
JG = 8                # j-chunks per psum-accumulation group
NGROUPS = JCH // JG   # 4
NPAIR = JG // 2       # chunk-pairs per group

_cache = {}


def _build_program(main_bf16: bool):
    import concourse.bacc as bacc
    import concourse.mybir as mybir
    import concourse.tile as tile
    from concourse.masks import make_identity

    f32 = mybir.dt.float32
    f32r = mybir.dt.float32r
    bf16 = mybir.dt.bfloat16
    mdt = mybir.dt.bfloat16 if main_bf16 else f32
    Prelu = mybir.ActivationFunctionType.Prelu
    Exp = mybir.ActivationFunctionType.Exp
    add = mybir.AluOpType.add
    mult = mybir.AluOpType.mult
    gps_every = int(os.environ.get("KERNEL_GPS_EVERY", "4"))
    pbuf_bufs = int(os.environ.get("KERNEL_PBUF", "7"))

    nc = bacc.Bacc(
        "TRN2",
        target_bir_lowering=False,
        debug=False,
        enable_asserts=False,
        num_devices=NCORES,
    )

    adjT = nc.dram_tensor("adjT", [N, RPC], bf16, kind="ExternalInput").ap()
    inpT = nc.dram_tensor("inpT", [IN, N], f32r, kind="ExternalInput").ap()
    wh = nc.dram_tensor("wh", [IN, D], f32r, kind="ExternalInput").ap()
    whT = nc.dram_tensor("whT", [D, IN], f32r, kind="ExternalInput").ap()
    uvh = nc.dram_tensor("uvh", [D, 2], f32r, kind="ExternalInput").ap()
    br = nc.dram_tensor("br", [2, D], f32, kind="ExternalInput").ap()
    pwh = nc.dram_tensor("pwh", [IN, D], f32r, kind="ExternalInput").ap()
    outb = nc.dram_tensor("outb", [RPC, D], f32, kind="ExternalOutput").ap()

    with tile.TileContext(nc) as tc:
        with tc.tile_pool(name="persist", bufs=1) as persist, \
             tc.tile_pool(name="adjp", bufs=2) as adjp, \
             tc.tile_pool(name="tmpp", bufs=2) as tmpp, \
             tc.tile_pool(name="pbufp", bufs=pbuf_bufs) as pbufp, \
             tc.tile_pool(name="epp", bufs=2) as epp, \
             tc.tile_pool(name="accp", bufs=2, space="PSUM") as accp:
            # supp/f12 are split per j-group so stage-2 consumers only wait
            # on the group's stage-1 writes (Tile deps are tile-granular)
            supp_g = [persist.tile([128, JG * (D + 1)], mdt, tag=f"supp{g}",
                                   name=f"supp{g}") for g in range(NGROUPS)]
            f12_g = [persist.tile([128, 2 * JG], f32, tag=f"f12{g}",
                                  name=f"f12{g}") for g in range(NGROUPS)]
            alpha_col = persist.tile([128, 1], f32)          # Prelu slope
            res = persist.tile([128, RPC], f32)              # residual+bias
            bias_bc = persist.tile([128, D], f32)            # (bias+proj_b)
            F2b = persist.tile([128, RPC], f32)              # f2 partition-bcast
            osum = persist.tile([128, ICH * (D + 1)], f32)   # out accumulator
            for g in range(NGROUPS):
                nc.vector.memset(supp_g[g], 1.0)
            nc.vector.memset(alpha_col, 0.2)

            # ---- stage 1: support/f1/f2/residual in one fused PE pass ----
            # rhs columns: [w1 | w2 | Wh | proj_w] so the f32r-able part
            # (support+residual, error enters linearly) is contiguous [2:258]
            # while f1/f2 (feeds exp, needs full fp32) is [0:2].
            with tc.tile_pool(name="s1c", bufs=1) as s1c, \
                 tc.tile_pool(name="s1p", bufs=2, space="PSUM") as s1p, \
                 tc.tile_pool(name="s1in", bufs=2) as s1in:
                whT_sb = s1c.tile([D, IN], f32r)
                nc.gpsimd.dma_start(out=whT_sb, in_=whT)
                uv_sb = s1c.tile([D, 2], f32r)
                nc.gpsimd.dma_start(out=uv_sb, in_=uvh)
                w12 = s1c.tile([128, 8], f32r)                # w1|w2 per k-chunk
                for kc in range(4):
                    wps = s1p.tile([128, 2], f32, tag="wps")
                    nc.tensor.matmul(
                        wps, whT_sb[:, kc * 128:(kc + 1) * 128], uv_sb,
                        start=True, stop=True,
                    )
                    nc.vector.tensor_copy(out=w12[:, 2 * kc:2 * kc + 2], in_=wps)

                rhs_sb = []
                for kc in range(4):
                    t = s1c.tile([128, 258], f32r, tag=f"rhs{kc}")
                    nc.vector.tensor_copy(
                        out=t[:, 0:2], in_=w12[:, 2 * kc:2 * kc + 2])
                    # issued from ACT: it is idle until stage 2 starts, and
                    # gpsimd's queue is busy with whT/uv (w12 critical path)
                    nc.scalar.dma_start(
                        out=t[:, 2:130], in_=wh[kc * 128:(kc + 1) * 128, :])
                    nc.scalar.dma_start(
                        out=t[:, 130:258], in_=pwh[kc * 128:(kc + 1) * 128, :])
                    rhs_sb.append(t)
                # (bias + proj_b) broadcast across partitions
                br2 = s1c.tile([1, 2 * D], f32)
                nc.scalar.dma_start(out=br2[0:1, 0:D], in_=br[0:1, :])
                nc.scalar.dma_start(out=br2[0:1, D:2 * D], in_=br[1:2, :])
                bsum = s1c.tile([1, D], f32)
                nc.vector.tensor_add(bsum, br2[0:1, 0:D], br2[0:1, D:2 * D])
                nc.gpsimd.partition_broadcast(bias_bc, bsum)

                f2row = s1c.tile([1, RPC], f32)

                # Load both own-row input blocks, then ALL f2 matmuls before
                # any support matmul: F2b is the gate for stage-2 activations,
                # so it must be first in the PE stream.
                it_blks = {}
                for blk in range(2):
                    it_blks[blk] = []
                    for kc in range(4):
                        t = s1in.tile([128, 8 * 128], f32r, tag=f"it{kc}")
                        nc.sync.dma_start(
                            out=t,
                            in_=inpT[kc * 128:(kc + 1) * 128,
                                     blk * 1024:(blk + 1) * 1024])
                        it_blks[blk].append(t)
                for blk in range(2):
                    for nchunk in range(2):
                        f2ps = s1p.tile([1, 512], f32, tag="f2ps")
                        for kc in range(4):
                            nc.tensor.matmul(
                                f2ps,
                                w12[:, 2 * kc + 1:2 * kc + 2],
                                it_blks[blk][kc][:, nchunk * 512:
                                                 (nchunk + 1) * 512],
                                start=(kc == 0), stop=(kc == 3),
                            )
                        # on ACT (idle here) so the F2b broadcast isn't
                        # queued behind stage-1's DVE copy stream
                        nc.scalar.copy(
                            out=f2row[0:1, blk * 1024 + nchunk * 512:
                                      blk * 1024 + (nchunk + 1) * 512],
                            in_=f2ps)
                nc.gpsimd.partition_broadcast(F2b, f2row)

                # inputsT streamed in 4 column-blocks of 1024 nodes
                for blk in range(4):
                    if blk < 2:
                        it = it_blks[blk]
                    else:
                        it = []
                        for kc in range(4):
                            t = s1in.tile([128, 8 * 128], f32r, tag=f"it{kc}")
                            nc.sync.dma_start(
                                out=t,
                                in_=inpT[kc * 128:(kc + 1) * 128,
                                         blk * 1024:(blk + 1) * 1024])
                            it.append(t)
                    for j8 in range(8):
                        jc = blk * 8 + j8
                        own = jc < ICH
                        ps = s1p.tile([128, 258], f32, tag="ps")
                        for kc in range(4):
                            lhsT = it[kc][:, j8 * 128:(j8 + 1) * 128]
                            nc.tensor.matmul(
                                ps, lhsT, rhs_sb[kc],
                                start=(kc == 0), stop=(kc == 3),
                            )
                        jg, jo = jc // JG, jc % JG
                        nc.vector.tensor_copy(
                            out=supp_g[jg][:, jo * 129:jo * 129 + 128],
                            in_=ps[:, 2:130])
                        nc.vector.tensor_copy(
                            out=f12_g[jg][:, 2 * jo:2 * jo + 2], in_=ps[:, 0:2])
                        if own:
                            nc.vector.scalar_tensor_tensor(
                                res[:, jc * 128:(jc + 1) * 128],
                                in0=ps[:, 130:258], scalar=0.0, in1=bias_bc,
                                op0=add, op1=add)

            # ---- stage 2: attention pairs + aggregation ----
            # Pairs of j-chunks share one [128, 2*RPC] tile so exp and the
            # mask multiply run double-width (amortizes fixed op costs).
            n_dve_prelu = int(os.environ.get("KERNEL_DVE_PRELU", "3"))
            dve_prelu = {int((i + 0.5) * (NGROUPS * NPAIR) / n_dve_prelu)
                         for i in range(n_dve_prelu)} if n_dve_prelu else set()
            # row-chunks per psum bank for the output accumulation
            ICB = 3
            ic_blocks = [list(range(s, min(s + ICB, ICH)))
                         for s in range(0, ICH, ICB)]
            for g in range(NGROUPS):
                pair_tiles = []
                for pr in range(NPAIR):
                    idx = g * NPAIR + pr
                    jc0 = g * JG + 2 * pr
                    adj_t = adjp.tile([128, 2 * RPC], bf16, tag="adj")
                    m_t = tmpp.tile([128, 2 * RPC], f32, tag="m")
                    for half in range(2):
                        jc = jc0 + half
                        jo = jc % JG
                        sl = slice(half * RPC, (half + 1) * RPC)
                        nc.sync.dma_start(
                            out=adj_t[:, sl],
                            in_=adjT[jc * 128:(jc + 1) * 128, :])
                        if idx in dve_prelu:
                            # leaky-relu on DVE to offload the ACT wall:
                            # s = f1+f2 (2x ts), then max(s, 0.2s) in place
                            nc.vector.tensor_scalar_add(
                                m_t[:, sl], F2b,
                                f12_g[g][:, 2 * jo:2 * jo + 1])
                        else:
                            nc.scalar.activation(
                                m_t[:, sl], F2b, Prelu,
                                bias=f12_g[g][:, 2 * jo:2 * jo + 1], scale=1.0,
                                alpha=alpha_col[:, 0:1])
                    if idx in dve_prelu:
                        nc.vector.scalar_tensor_tensor(
                            m_t, in0=m_t, scalar=0.2, in1=m_t,
                            op0=mult, op1=mybir.AluOpType.max)
                    # exp writes bf16 straight into the p tile; the bf16
                    # adjacency mask is applied in place at DVE 2x rate
                    p_t = pbufp.tile([128, 2 * RPC], mdt, tag="pbuf")
                    nc.scalar.activation(p_t, m_t, Exp)
                    eng = nc.gpsimd if idx % gps_every == 0 else nc.vector
                    eng.tensor_mul(p_t, adj_t, p_t)
                    pair_tiles.append(p_t)
                # consume in two half-groups (pairs 0-1, then 2-3) so the
                # matmuls start before the later pairs' masks finish and
                # pbuf slots free earlier; ICB row-chunks share one psum
                # bank so one flush-add covers ICB chunks
                for hg in range(2):
                    for icb in ic_blocks:
                        acc = accp.tile([128, ICB * (D + 1)], f32, tag="acc")
                        for i3, ic in enumerate(icb):
                            asl = slice(i3 * 129, i3 * 129 + 129)
                            for jj in range(hg * 4, hg * 4 + 4):
                                lhsT = pair_tiles[jj // 2][
                                    :, (jj % 2) * RPC + ic * 128:
                                       (jj % 2) * RPC + (ic + 1) * 128]
                                nc.tensor.matmul(
                                    acc[:, asl], lhsT,
                                    supp_g[g][:, jj * 129:(jj + 1) * 129],
                                    start=(jj == hg * 4),
                                    stop=(jj == hg * 4 + 3),
                                )
                        W3 = len(icb) * 129
                        dst = osum[:, icb[0] * 129:icb[0] * 129 + W3]
                        if g == 0 and hg == 0:
                            # +1e-30 guards the (measure-zero) all-masked-row
                            # 0/0 case; harmless elsewhere
                            nc.vector.tensor_scalar_add(
                                dst, acc[:, 0:W3], 1e-30)
                        else:
                            nc.vector.tensor_add(dst, dst, acc[:, 0:W3])
                        if g == NGROUPS - 1 and hg == 1:
                            # epilogue inline: normalize + residual + store
                            rc = epp.tile([128, ICB], f32, tag="rc")
                            osr = osum.rearrange("p (i c) -> p i c", c=D + 1)
                            nc.vector.reciprocal(
                                rc[:, 0:len(icb)],
                                osr[:, icb[0]:icb[0] + len(icb), D])
                            for i3, ic in enumerate(icb):
                                of = epp.tile([128, D], f32, tag="of")
                                nc.vector.scalar_tensor_tensor(
                                    of, in0=osum[:, ic * 129:ic * 129 + 128],
                                    scalar=rc[:, i3:i3 + 1],
                                    in1=res[:, ic * 128:(ic + 1) * 128],
                                    op0=mult, op1=add)
                                nc.sync.dma_start(
                                    out=outb[ic * 128:(ic + 1) * 128, :],
                                    in_=of)

    nc.compile()
    return nc


def _get_program():
    main_bf16 = os.environ.get("KERNEL_MAIN_BF16", "1") == "1"
    key = ("prog", main_bf16,
           os.environ.get("KERNEL_GPS_EVERY", "4"),
           os.environ.get("KERNEL_PBUF", "7"),
           os.environ.get("KERNEL_DVE_PRELU", "3"))
    if key not in _cache:
        _cache[key] = _build_program(main_bf16)
    return _cache[key]


def kernel(inputs, adjacency, weight, weight_u, weight_v, bias, proj_w, proj_b):
    from concourse.bass_utils import run_bass_kernel_spmd

    inputs = np.ascontiguousarray(np.asarray(inputs, np.float32))
    adjacency = np.asarray(adjacency, np.float32)
    weight = np.asarray(weight, np.float32)
    weight_u = np.asarray(weight_u, np.float32)
    weight_v = np.asarray(weight_v, np.float32)
    bias = np.asarray(bias, np.float32).reshape(1, H * D)
    proj_w = np.asarray(proj_w, np.float32)
    proj_b = np.asarray(proj_b, np.float32).reshape(H * D)

    nc = _get_program()

    in_maps = []
    for c in range(NCORES):
        h = c // 2
        r0 = (c % 2) * RPC
        hs = slice(h * D, (h + 1) * D)
        # rolled node order: own query rows first
        rolled_inputs = np.roll(inputs, -r0, axis=0)
        inpT_ext = np.ascontiguousarray(rolled_inputs.T)
        adjT_c = np.ascontiguousarray(
            np.roll(adjacency[r0:r0 + RPC, :], -r0, axis=1).T
        ).astype(ml_dtypes.bfloat16)  # exact: adjacency is 0.0/1.0
        in_maps.append({
            "adjT": adjT_c,
            "inpT": inpT_ext,
            "wh": np.ascontiguousarray(weight[:, hs]),
            "whT": np.ascontiguousarray(weight[:, hs].T),
            "uvh": np.ascontiguousarray(
                np.concatenate([weight_u[h], weight_v[h]], axis=1)),
            "br": np.ascontiguousarray(
                np.stack([bias[0, hs], proj_b[hs]], axis=0)),
            "pwh": np.ascontiguousarray(proj_w[:, hs]),
        })

    trace = os.environ.get("KERNEL_TRACE", "0") == "1"
    results = run_bass_kernel_spmd(
        nc, in_maps, core_ids=list(range(NCORES)), trace=trace)
    _cache["last_results"] = results

    out = np.empty((N, H * D), np.float32)
    for c in range(NCORES):
        h = c // 2
        r0 = (c % 2) * RPC
        out[r0:r0 + RPC, h * D:(h + 1) * D] = results.results[c]["outb"]
    return out


# revision 42
# speedup vs baseline: 1.0421x; 1.0421x over previous
"""GAT-style dense-mask attention (gnn_message_passing) on 8 trn2 cores.

Sharding: core c owns head h=c//2 and query rows [r0, r0+2048), r0=(c%2)*2048.
Inputs are pre-sliced/rolled on host so all 8 cores run one identical SPMD
program; outputs are [2048,128] blocks reassembled on host.

Math (per core, node order rolled so own rows come first):
  support = X @ Wh                      [4096, 128]
  f1 = X @ (Wh @ u), f2 = X @ (Wh @ v)  [4096]
  p[j,i] = adj[i,j] * exp(prelu_0.2(f1[j]+f2[i]))     (j on partitions)
  out[i,:] = (p.T @ [support|1])[:, :128] / (p.T @ [support|1])[:, 128]
             + X[i] @ proj_w_h + bias_h + proj_b_h
The mask multiply uses exp underflow (adj=0 -> p=0) instead of the -1e30
trick, which is exactly equivalent because softmax is shift-invariant.

Schedule: stage-2 pools are allocated before stage-1's streaming input pool
so their SBUF ranges are disjoint — attention tiles (ACT prelu/exp, DVE/GPS
mask) overlap the tail of the support pass on PE.
"""

import os

import ml_dtypes
import numpy as np

N = 4096
IN = 512
D = 128
H = 4
NCORES = 8
RPC = N // 2          # query rows per core
JCH = N // 128        # 32 source-node chunks
ICH = RPC // 128      # 16 query-row chunks
JG = 8                # j-chunks per psum-accumulation group
NGROUPS = JCH // JG   # 4
NPAIR = JG // 2       # chunk-pairs per group

_cache = {}


def _build_program(main_bf16: bool):
    import concourse.bacc as bacc
    import concourse.mybir as mybir
    import concourse.tile as tile
    from concourse.masks import make_identity

    f32 = mybir.dt.float32
    f32r = mybir.dt.float32r
    bf16 = mybir.dt.bfloat16
    mdt = mybir.dt.bfloat16 if main_bf16 else f32
    Prelu = mybir.ActivationFunctionType.Prelu
    Exp = mybir.ActivationFunctionType.Exp
    add = mybir.AluOpType.add
    mult = mybir.AluOpType.mult
    gps_every = int(os.environ.get("KERNEL_GPS_EVERY", "4"))
    pbuf_bufs = int(os.environ.get("KERNEL_PBUF", "7"))

    nc = bacc.Bacc(
        "TRN2",
        target_bir_lowering=False,
        debug=False,
        enable_asserts=False,
        num_devices=NCORES,
    )

    adjT = nc.dram_tensor("adjT", [N, RPC], bf16, kind="ExternalInput").ap()
    inpT = nc.dram_tensor("inpT", [IN, N], f32r, kind="ExternalInput").ap()
    wh = nc.dram_tensor("wh", [IN, D], f32r, kind="ExternalInput").ap()
    whT = nc.dram_tensor("whT", [D, IN], f32r, kind="ExternalInput").ap()
    uvh = nc.dram_tensor("uvh", [D, 2], f32r, kind="ExternalInput").ap()
    br = nc.dram_tensor("br", [2, D], f32, kind="ExternalInput").ap()
    pwh = nc.dram_tensor("pwh", [IN, D], f32r, kind="ExternalInput").ap()
    outb = nc.dram_tensor("outb", [RPC, D], f32, kind="ExternalOutput").ap()

    with tile.TileContext(nc) as tc:
        with tc.tile_pool(name="persist", bufs=1) as persist, \
             tc.tile_pool(name="adjp", bufs=2) as adjp, \
             tc.tile_pool(name="tmpp", bufs=2) as tmpp, \
             tc.tile_pool(name="pbufp", bufs=pbuf_bufs) as pbufp, \
             tc.tile_pool(name="epp", bufs=2) as epp, \
             tc.tile_pool(name="accp", bufs=2, space="PSUM") as accp:
            # supp/f12 are split per j-group so stage-2 consumers only wait
            # on the group's stage-1 writes (Tile deps are tile-granular)
            supp_g = [persist.tile([128, JG * (D + 1)], mdt, tag=f"supp{g}",
                                   name=f"supp{g}") for g in range(NGROUPS)]
            f12_g = [persist.tile([128, 2 * JG], f32, tag=f"f12{g}",
                                  name=f"f12{g}") for g in range(NGROUPS)]
            alpha_col = persist.tile([128, 1], f32)          # Prelu slope
            res = persist.tile([128, RPC], f32)              # residual+bias
            bias_bc = persist.tile([128, D], f32)            # (bias+proj_b)
            F2b = persist.tile([128, RPC], f32)              # f2 partition-bcast
            osum = persist.tile([128, ICH * (D + 1)], f32)   # out accumulator
            for g in range(NGROUPS):
                nc.vector.memset(supp_g[g], 1.0)
            nc.vector.memset(alpha_col, 0.2)

            # ---- stage 1: support/f1/f2/residual in one fused PE pass ----
            # rhs columns: [w1 | w2 | Wh | proj_w] so the f32r-able part
            # (support+residual, error enters linearly) is contiguous [2:258]
            # while f1/f2 (feeds exp, needs full fp32) is [0:2].
            with tc.tile_pool(name="s1c", bufs=1) as s1c, \
                 tc.tile_pool(name="s1p", bufs=2, space="PSUM") as s1p, \
                 tc.tile_pool(name="s1in", bufs=2) as s1in:
                whT_sb = s1c.tile([D, IN], f32r)
                nc.gpsimd.dma_start(out=whT_sb, in_=whT)
                uv_sb = s1c.tile([D, 2], f32r)
                nc.gpsimd.dma_start(out=uv_sb, in_=uvh)
                w12 = s1c.tile([128, 8], f32r)                # w1|w2 per k-chunk
                for kc in range(4):
                    wps = s1p.tile([128, 2], f32, tag="wps")
                    nc.tensor.matmul(
                        wps, whT_sb[:, kc * 128:(kc + 1) * 128], uv_sb,
                        start=True, stop=True,
                    )
                    nc.vector.tensor_copy(out=w12[:, 2 * kc:2 * kc + 2], in_=wps)

                rhs_sb = []
                for kc in range(4):
                    t = s1c.tile([128, 258], f32r, tag=f"rhs{kc}")
                    nc.vector.tensor_copy(
                        out=t[:, 0:2], in_=w12[:, 2 * kc:2 * kc + 2])
                    # issued from ACT: it is idle until stage 2 starts, and
                    # gpsimd's queue is busy with whT/uv (w12 critical path)
                    nc.scalar.dma_start(
                        out=t[:, 2:130], in_=wh[kc * 128:(kc + 1) * 128, :])
                    nc.scalar.dma_start(
                        out=t[:, 130:258], in_=pwh[kc * 128:(kc + 1) * 128, :])
                    rhs_sb.append(t)
                # (bias + proj_b) broadcast across partitions
                br2 = s1c.tile([1, 2 * D], f32)
                nc.scalar.dma_start(out=br2[0:1, 0:D], in_=br[0:1, :])
                nc.scalar.dma_start(out=br2[0:1, D:2 * D], in_=br[1:2, :])
                bsum = s1c.tile([1, D], f32)
                nc.vector.tensor_add(bsum, br2[0:1, 0:D], br2[0:1, D:2 * D])
                nc.gpsimd.partition_broadcast(bias_bc, bsum)

                f2row = s1c.tile([1, RPC], f32)

                # Load both own-row input blocks, then ALL f2 matmuls before
                # any support matmul: F2b is the gate for stage-2 activations,
                # so it must be first in the PE stream.
                it_blks = {}
                for blk in range(2):
                    it_blks[blk] = []
                    for kc in range(4):
                        t = s1in.tile([128, 8 * 128], f32r, tag=f"it{kc}")
                        nc.sync.dma_start(
                            out=t,
                            in_=inpT[kc * 128:(kc + 1) * 128,
                                     blk * 1024:(blk + 1) * 1024])
                        it_blks[blk].append(t)
                for blk in range(2):
                    for nchunk in range(2):
                        f2ps = s1p.tile([1, 512], f32, tag="f2ps")
                        for kc in range(4):
                            nc.tensor.matmul(
                                f2ps,
                                w12[:, 2 * kc + 1:2 * kc + 2],
                                it_blks[blk][kc][:, nchunk * 512:
                                                 (nchunk + 1) * 512],
                                start=(kc == 0), stop=(kc == 3),
                            )
                        # on ACT (idle here) so the F2b broadcast isn't
                        # queued behind stage-1's DVE copy stream
                        nc.scalar.copy(
                            out=f2row[0:1, blk * 1024 + nchunk * 512:
                                      blk * 1024 + (nchunk + 1) * 512],
                            in_=f2ps)
                nc.gpsimd.partition_broadcast(F2b, f2row)

                # inputsT streamed in 4 column-blocks of 1024 nodes
                for blk in range(4):
                    if blk < 2:
                        it = it_blks[blk]
                    else:
                        it = []
                        for kc in range(4):
                            t = s1in.tile([128, 8 * 128], f32r, tag=f"it{kc}")
                            nc.sync.dma_start(
                                out=t,
                                in_=inpT[kc * 128:(kc + 1) * 128,
                                         blk * 1024:(blk + 1) * 1024])
                            it.append(t)
                    for j8 in range(8):
                        jc = blk * 8 + j8
                        own = jc < ICH
                        ps = s1p.tile([128, 258], f32, tag="ps")
                        for kc in range(4):
                            lhsT = it[kc][:, j8 * 128:(j8 + 1) * 128]
                            nc.tensor.matmul(
                                ps, lhsT, rhs_sb[kc],
                                start=(kc == 0), stop=(kc == 3),
                            )
                        jg, jo = jc // JG, jc % JG
                        nc.vector.tensor_copy(
                            out=supp_g[jg][:, jo * 129:jo * 129 + 128],
                            in_=ps[:, 2:130])
                        nc.vector.tensor_copy(
                            out=f12_g[jg][:, 2 * jo:2 * jo + 2], in_=ps[:, 0:2])
                        if own:
                            nc.vector.scalar_tensor_tensor(
                                res[:, jc * 128:(jc + 1) * 128],
                                in0=ps[:, 130:258], scalar=0.0, in1=bias_bc,
                                op0=add, op1=add)

            # ---- stage 2: attention pairs + aggregation ----
            # Pairs of j-chunks share one [128, 2*RPC] tile so exp and the
            # mask multiply run double-width (amortizes fixed op costs).
            n_dve_prelu = int(os.environ.get("KERNEL_DVE_PRELU", "0"))
            dve_prelu = {int((i + 0.5) * (NGROUPS * NPAIR) / n_dve_prelu)
                         for i in range(n_dve_prelu)} if n_dve_prelu else set()
            # row-chunks per psum bank for the output accumulation
            ICB = 3
            ic_blocks = [list(range(s, min(s + ICB, ICH)))
                         for s in range(0, ICH, ICB)]
            for g in range(NGROUPS):
                pair_tiles = []
                for pr in range(NPAIR):
                    idx = g * NPAIR + pr
                    jc0 = g * JG + 2 * pr
                    adj_t = adjp.tile([128, 2 * RPC], bf16, tag="adj")
                    m_t = tmpp.tile([128, 2 * RPC], f32, tag="m")
                    for half in range(2):
                        jc = jc0 + half
                        jo = jc % JG
                        sl = slice(half * RPC, (half + 1) * RPC)
                        nc.sync.dma_start(
                            out=adj_t[:, sl],
                            in_=adjT[jc * 128:(jc + 1) * 128, :])
                        if idx in dve_prelu:
                            # leaky-relu on DVE to offload the ACT wall:
                            # s = f1+f2 (2x ts), then max(s, 0.2s) in place
                            nc.vector.tensor_scalar_add(
                                m_t[:, sl], F2b,
                                f12_g[g][:, 2 * jo:2 * jo + 1])
                        else:
                            nc.scalar.activation(
                                m_t[:, sl], F2b, Prelu,
                                bias=f12_g[g][:, 2 * jo:2 * jo + 1], scale=1.0,
                                alpha=alpha_col[:, 0:1])
                    if idx in dve_prelu:
                        nc.vector.scalar_tensor_tensor(
                            m_t, in0=m_t, scalar=0.2, in1=m_t,
                            op0=mult, op1=mybir.AluOpType.max)
                    # exp writes bf16 straight into the p tile; the bf16
                    # adjacency mask is applied in place at DVE 2x rate
                    p_t = pbufp.tile([128, 2 * RPC], mdt, tag="pbuf")
                    nc.scalar.activation(p_t, m_t, Exp)
                    eng = nc.gpsimd if idx % gps_every == 0 else nc.vector
                    eng.tensor_mul(p_t, adj_t, p_t)
                    pair_tiles.append(p_t)
                # consume in two half-groups (pairs 0-1, then 2-3) so the
                # matmuls start before the later pairs' masks finish and
                # pbuf slots free earlier; ICB row-chunks share one psum
                # bank so one flush-add covers ICB chunks
                for hg in range(2):
                    for icb in ic_blocks:
                        acc = accp.tile([128, ICB * (D + 1)], f32, tag="acc")
                        for i3, ic in enumerate(icb):
                            asl = slice(i3 * 129, i3 * 129 + 129)
                            for jj in range(hg * 4, hg * 4 + 4):
                                lhsT = pair_tiles[jj // 2][
                                    :, (jj % 2) * RPC + ic * 128:
                                       (jj % 2) * RPC + (ic + 1) * 128]
                                nc.tensor.matmul(
                                    acc[:, asl], lhsT,
                                    supp_g[g][:, jj * 129:(jj + 1) * 129],
                                    start=(jj == hg * 4),
                                    stop=(jj == hg * 4 + 3),
                                )
                        W3 = len(icb) * 129
                        dst = osum[:, icb[0] * 129:icb[0] * 129 + W3]
                        if g == 0 and hg == 0:
                            # +1e-30 guards the (measure-zero) all-masked-row
                            # 0/0 case; harmless elsewhere
                            nc.vector.tensor_scalar_add(
                                dst, acc[:, 0:W3], 1e-30)
                        else:
                            nc.vector.tensor_add(dst, dst, acc[:, 0:W3])
                        if g == NGROUPS - 1 and hg == 1:
                            # epilogue inline: normalize + residual + store
                            rc = epp.tile([128, ICB], f32, tag="rc")
                            osr = osum.rearrange("p (i c) -> p i c", c=D + 1)
                            nc.vector.reciprocal(
                                rc[:, 0:len(icb)],
                                osr[:, icb[0]:icb[0] + len(icb), D])
                            for i3, ic in enumerate(icb):
                                of = epp.tile([128, D], f32, tag="of")
                                nc.vector.scalar_tensor_tensor(
                                    of, in0=osum[:, ic * 129:ic * 129 + 128],
                                    scalar=rc[:, i3:i3 + 1],
                                    in1=res[:, ic * 128:(ic + 1) * 128],
                                    op0=mult, op1=add)
                                nc.sync.dma_start(
                                    out=outb[ic * 128:(ic + 1) * 128, :],
                                    in_=of)

    nc.compile()
    return nc


def _get_program():
    main_bf16 = os.environ.get("KERNEL_MAIN_BF16", "1") == "1"
    key = ("prog", main_bf16,
           os.environ.get("KERNEL_GPS_EVERY", "4"),
           os.environ.get("KERNEL_PBUF", "7"),
           os.environ.get("KERNEL_DVE_PRELU", "0"))
    if key not in _cache:
        _cache[key] = _build_program(main_bf16)
    return _cache[key]


def kernel(inputs, adjacency, weight, weight_u, weight_v, bias, proj_w, proj_b):
    from concourse.bass_utils import run_bass_kernel_spmd

    inputs = np.ascontiguousarray(np.asarray(inputs, np.float32))
    adjacency = np.asarray(adjacency, np.float32)
    weight = np.asarray(weight, np.float32)
    weight_u = np.asarray(weight_u, np.float32)
    weight_v = np.asarray(weight_v, np.float32)
    bias = np.asarray(bias, np.float32).reshape(1, H * D)
    proj_w = np.asarray(proj_w, np.float32)
    proj_b = np.asarray(proj_b, np.float32).reshape(H * D)

    nc = _get_program()

    in_maps = []
    for c in range(NCORES):
        h = c // 2
        r0 = (c % 2) * RPC
        hs = slice(h * D, (h + 1) * D)
        # rolled node order: own query rows first
        rolled_inputs = np.roll(inputs, -r0, axis=0)
        inpT_ext = np.ascontiguousarray(rolled_inputs.T)
        adjT_c = np.ascontiguousarray(
            np.roll(adjacency[r0:r0 + RPC, :], -r0, axis=1).T
        ).astype(ml_dtypes.bfloat16)  # exact: adjacency is 0.0/1.0
        in_maps.append({
            "adjT": adjT_c,
            "inpT": inpT_ext,
            "wh": np.ascontiguousarray(weight[:, hs]),
            "whT": np.ascontiguousarray(weight[:, hs].T),
            "uvh": np.ascontiguousarray(
                np.concatenate([weight_u[h], weight_v[h]], axis=1)),
            "br": np.ascontiguousarray(
                np.stack([bias[0, hs], proj_b[hs]], axis=0)),
            "pwh": np.ascontiguousarray(proj_w[:, hs]),
        })

    trace = os.environ.get("KERNEL_TRACE", "0") == "1"
    results = run_bass_kernel_spmd(
        nc, in_maps, core_ids=list(range(NCORES)), trace=trace)
    _cache["last_results"] = results

    out = np.empty((N, H * D), np.float32)
    for c in range(NCORES):
        h = c // 2
        r0 = (c % 2) * RPC
        out[r0:r0 + RPC, h * D:(h + 1) * D] = results.results[c]["outb"]
    return out


# revision 48
# speedup vs baseline: 1.0509x; 1.0085x over previous
"""GAT-style dense-mask attention (gnn_message_passing) on 8 trn2 cores.

Sharding: core c owns head h=c//2 and query rows [r0, r0+2048), r0=(c%2)*2048.
Inputs are pre-sliced/rolled on host so all 8 cores run one identical SPMD
program; outputs are [2048,128] blocks reassembled on host.

Math (per core, node order rolled so own rows come first):
  support = X @ Wh                      [4096, 128]
  f1 = X @ (Wh @ u), f2 = X @ (Wh @ v)  [4096]
  p[j,i] = adj[i,j] * exp(prelu_0.2(f1[j]+f2[i]))     (j on partitions)
  out[i,:] = (p.T @ [support|1])[:, :128] / (p.T @ [support|1])[:, 128]
             + X[i] @ proj_w_h + bias_h + proj_b_h
The mask multiply uses exp underflow (adj=0 -> p=0) instead of the -1e30
trick, which is exactly equivalent because softmax is shift-invariant.

Schedule: stage-2 pools are allocated before stage-1's streaming input pool
so their SBUF ranges are disjoint — attention tiles (ACT prelu/exp, DVE/GPS
mask) overlap the tail of the support pass on PE.
"""

import os

import ml_dtypes
import numpy as np

N = 4096
IN = 512
D = 128
H = 4
NCORES = 8
RPC = N // 2          # query rows per core
JCH = N // 128        # 32 source-node chunks
ICH = RPC // 128      # 16 query-row chunks
JG = 8                # j-chunks per psum-accumulation group
NGROUPS = JCH // JG   # 4
NPAIR = JG // 2       # chunk-pairs per group

_cache = {}


def _build_program(main_bf16: bool):
    import concourse.bacc as bacc
    import concourse.mybir as mybir
    import concourse.tile as tile
    from concourse.masks import make_identity

    f32 = mybir.dt.float32
    f32r = mybir.dt.float32r
    bf16 = mybir.dt.bfloat16
    mdt = mybir.dt.bfloat16 if main_bf16 else f32
    Prelu = mybir.ActivationFunctionType.Prelu
    Exp = mybir.ActivationFunctionType.Exp
    add = mybir.AluOpType.add
    mult = mybir.AluOpType.mult
    gps_every = int(os.environ.get("KERNEL_GPS_EVERY", "4"))
    pbuf_bufs = int(os.environ.get("KERNEL_PBUF", "7"))

    nc = bacc.Bacc(
        "TRN2",
        target_bir_lowering=False,
        debug=False,
        enable_asserts=False,
        num_devices=NCORES,
    )

    adjT = nc.dram_tensor("adjT", [N, RPC], bf16, kind="ExternalInput").ap()
    inpT = nc.dram_tensor("inpT", [IN, N], f32r, kind="ExternalInput").ap()
    wh = nc.dram_tensor("wh", [IN, D], f32r, kind="ExternalInput").ap()
    whT = nc.dram_tensor("whT", [D, IN], f32r, kind="ExternalInput").ap()
    uvh = nc.dram_tensor("uvh", [D, 2], f32r, kind="ExternalInput").ap()
    br = nc.dram_tensor("br", [2, D], f32, kind="ExternalInput").ap()
    pwh = nc.dram_tensor("pwh", [IN, D], f32r, kind="ExternalInput").ap()
    outb = nc.dram_tensor("outb", [RPC, D], f32, kind="ExternalOutput").ap()

    with tile.TileContext(nc) as tc:
        with tc.tile_pool(name="persist", bufs=1) as persist, \
             tc.tile_pool(name="adjp", bufs=2) as adjp, \
             tc.tile_pool(name="tmpp", bufs=2) as tmpp, \
             tc.tile_pool(name="pbufp", bufs=pbuf_bufs) as pbufp, \
             tc.tile_pool(name="epp", bufs=2) as epp, \
             tc.tile_pool(name="accp", bufs=2, space="PSUM") as accp:
            # supp/f12 are split per j-group so stage-2 consumers only wait
            # on the group's stage-1 writes (Tile deps are tile-granular)
            supp_g = [persist.tile([128, JG * (D + 1)], mdt, tag=f"supp{g}",
                                   name=f"supp{g}") for g in range(NGROUPS)]
            f12_g = [persist.tile([128, JG], f32, tag=f"f12{g}",
                                  name=f"f12{g}") for g in range(NGROUPS)]
            alpha_col = persist.tile([128, 1], f32)          # Prelu slope
            res = persist.tile([128, RPC], f32)              # residual+bias
            bias_bc = persist.tile([128, D], f32)            # (bias+proj_b)
            F2b = persist.tile([128, RPC], f32)              # f2 partition-bcast
            osum = persist.tile([128, ICH * (D + 1)], f32)   # out accumulator
            for g in range(NGROUPS):
                nc.vector.memset(supp_g[g], 1.0)
            nc.vector.memset(alpha_col, 0.2)

            # ---- stage 1: support/f1/f2/residual in one fused PE pass ----
            # rhs columns: [w1 | w2 | Wh | proj_w] so the f32r-able part
            # (support+residual, error enters linearly) is contiguous [2:258]
            # while f1/f2 (feeds exp, needs full fp32) is [0:2].
            with tc.tile_pool(name="s1c", bufs=1) as s1c, \
                 tc.tile_pool(name="s1p", bufs=2, space="PSUM") as s1p, \
                 tc.tile_pool(name="s1in", bufs=2) as s1in:
                whT_sb = s1c.tile([D, IN], f32r)
                nc.gpsimd.dma_start(out=whT_sb, in_=whT)
                uv_sb = s1c.tile([D, 2], f32r)
                nc.gpsimd.dma_start(out=uv_sb, in_=uvh)
                w12 = s1c.tile([128, 8], f32r)                # w1|w2 per k-chunk
                for kc in range(4):
                    wps = s1p.tile([128, 2], f32, tag="f2ps")
                    nc.tensor.matmul(
                        wps, whT_sb[:, kc * 128:(kc + 1) * 128], uv_sb,
                        start=True, stop=True,
                    )
                    nc.vector.tensor_copy(out=w12[:, 2 * kc:2 * kc + 2], in_=wps)

                rhs_sb = []
                for kc in range(4):
                    t = s1c.tile([128, 258], f32r, tag=f"rhs{kc}")  # col 257 pad
                    nc.vector.memset(t[:, 257:258].bitcast(f32), 0.0)
                    nc.vector.tensor_copy(
                        out=t[:, 0:1], in_=w12[:, 2 * kc:2 * kc + 1])
                    # issued from ACT: it is idle until stage 2 starts, and
                    # gpsimd's queue is busy with whT/uv (w12 critical path)
                    nc.scalar.dma_start(
                        out=t[:, 1:129], in_=wh[kc * 128:(kc + 1) * 128, :])
                    nc.scalar.dma_start(
                        out=t[:, 129:257], in_=pwh[kc * 128:(kc + 1) * 128, :])
                    rhs_sb.append(t)
                # (bias + proj_b) broadcast across partitions
                br2 = s1c.tile([1, 2 * D], f32)
                nc.scalar.dma_start(out=br2[0:1, 0:D], in_=br[0:1, :])
                nc.scalar.dma_start(out=br2[0:1, D:2 * D], in_=br[1:2, :])
                bsum = s1c.tile([1, D], f32)
                nc.vector.tensor_add(bsum, br2[0:1, 0:D], br2[0:1, D:2 * D])
                nc.gpsimd.partition_broadcast(bias_bc, bsum)

                f2row = s1c.tile([1, RPC], f32)

                # Load both own-row input blocks, then ALL f2 matmuls before
                # any support matmul: F2b is the gate for stage-2 activations,
                # so it must be first in the PE stream.
                it_blks = {}
                for blk in range(2):
                    it_blks[blk] = []
                    for kc in range(4):
                        t = s1in.tile([128, 8 * 128], f32r, tag=f"it{kc}")
                        nc.sync.dma_start(
                            out=t,
                            in_=inpT[kc * 128:(kc + 1) * 128,
                                     blk * 1024:(blk + 1) * 1024])
                        it_blks[blk].append(t)
                for blk in range(2):
                    for nchunk in range(2):
                        f2ps = s1p.tile([1, 512], f32, tag="f2ps")
                        for kc in range(4):
                            nc.tensor.matmul(
                                f2ps,
                                w12[:, 2 * kc + 1:2 * kc + 2],
                                it_blks[blk][kc][:, nchunk * 512:
                                                 (nchunk + 1) * 512],
                                start=(kc == 0), stop=(kc == 3),
                            )
                        # on ACT (idle here) so the F2b broadcast isn't
                        # queued behind stage-1's DVE copy stream
                        nc.scalar.copy(
                            out=f2row[0:1, blk * 1024 + nchunk * 512:
                                      blk * 1024 + (nchunk + 1) * 512],
                            in_=f2ps)
                nc.gpsimd.partition_broadcast(F2b, f2row)

                # inputsT streamed in 4 column-blocks of 1024 nodes
                for blk in range(4):
                    if blk < 2:
                        it = it_blks[blk]
                    else:
                        it = []
                        for kc in range(4):
                            t = s1in.tile([128, 8 * 128], f32r, tag=f"it{kc}")
                            nc.sync.dma_start(
                                out=t,
                                in_=inpT[kc * 128:(kc + 1) * 128,
                                         blk * 1024:(blk + 1) * 1024])
                            it.append(t)
                    for jp in range(4):       # pairs of j-chunks
                        jc = blk * 8 + 2 * jp
                        own = jc < ICH
                        # halves bank-aligned: matmul out must stay in a bank
                        ps = s1p.tile([128, 2, 512], f32, tag="ps")
                        for half in range(2):
                            for kc in range(4):
                                lhsT = it[kc][:, (2 * jp + half) * 128:
                                              (2 * jp + half + 1) * 128]
                                nc.tensor.matmul(
                                    ps[:, half, 0:258], lhsT, rhs_sb[kc],
                                    start=(kc == 0), stop=(kc == 3),
                                )
                        jg, jo = jc // JG, jc % JG
                        # strided pair-copies: one DVE op covers both chunks
                        so = supp_g[jg][:, jo * 129:(jo + 2) * 129].rearrange(
                            "p (c w) -> p c w", c=2)[:, :, 0:128]
                        nc.vector.tensor_copy(out=so, in_=ps[:, :, 1:129])
                        nc.vector.tensor_copy(
                            out=f12_g[jg][:, jo:jo + 2], in_=ps[:, :, 0:1])
                        if own:
                            for half in range(2):
                                nc.vector.scalar_tensor_tensor(
                                    res[:, (jc + half) * 128:
                                        (jc + half + 1) * 128],
                                    in0=ps[:, half, 129:257], scalar=0.0,
                                    in1=bias_bc, op0=add, op1=add)

            # ---- stage 2: attention pairs + aggregation ----
            # Pairs of j-chunks share one [128, 2*RPC] tile so exp and the
            # mask multiply run double-width (amortizes fixed op costs).
            n_dve_prelu = int(os.environ.get("KERNEL_DVE_PRELU", "0"))
            dve_prelu = {int((i + 0.5) * (NGROUPS * NPAIR) / n_dve_prelu)
                         for i in range(n_dve_prelu)} if n_dve_prelu else set()
            # row-chunks per psum bank for the output accumulation
            ICB = 3
            ic_blocks = [list(range(s, min(s + ICB, ICH)))
                         for s in range(0, ICH, ICB)]
            for g in range(NGROUPS):
                pair_tiles = []
                for pr in range(NPAIR):
                    idx = g * NPAIR + pr
                    jc0 = g * JG + 2 * pr
                    adj_t = adjp.tile([128, 2 * RPC], bf16, tag="adj")
                    m_t = tmpp.tile([128, 2 * RPC], f32, tag="m")
                    for half in range(2):
                        jc = jc0 + half
                        jo = jc % JG
                        sl = slice(half * RPC, (half + 1) * RPC)
                        nc.sync.dma_start(
                            out=adj_t[:, sl],
                            in_=adjT[jc * 128:(jc + 1) * 128, :])
                        if idx in dve_prelu:
                            # leaky-relu on DVE to offload the ACT wall:
                            # s = f1+f2 (2x ts), then max(s, 0.2s) in place
                            nc.vector.tensor_scalar_add(
                                m_t[:, sl], F2b,
                                f12_g[g][:, jo:jo + 1])
                        else:
                            nc.scalar.activation(
                                m_t[:, sl], F2b, Prelu,
                                bias=f12_g[g][:, jo:jo + 1], scale=1.0,
                                alpha=alpha_col[:, 0:1])
                    if idx in dve_prelu:
                        nc.vector.scalar_tensor_tensor(
                            m_t, in0=m_t, scalar=0.2, in1=m_t,
                            op0=mult, op1=mybir.AluOpType.max)
                    # exp writes bf16 straight into the p tile; the bf16
                    # adjacency mask is applied in place at DVE 2x rate
                    p_t = pbufp.tile([128, 2 * RPC], mdt, tag="pbuf")
                    nc.scalar.activation(p_t, m_t, Exp)
                    eng = nc.gpsimd if idx % gps_every == 0 else nc.vector
                    eng.tensor_mul(p_t, adj_t, p_t)
                    pair_tiles.append(p_t)
                # consume in two half-groups (pairs 0-1, then 2-3) so the
                # matmuls start before the later pairs' masks finish and
                # pbuf slots free earlier; ICB row-chunks share one psum
                # bank so one flush-add covers ICB chunks
                for hg in range(2):
                    for icb in ic_blocks:
                        acc = accp.tile([128, ICB * (D + 1)], f32, tag="acc")
                        for i3, ic in enumerate(icb):
                            asl = slice(i3 * 129, i3 * 129 + 129)
                            for jj in range(hg * 4, hg * 4 + 4):
                                lhsT = pair_tiles[jj // 2][
                                    :, (jj % 2) * RPC + ic * 128:
                                       (jj % 2) * RPC + (ic + 1) * 128]
                                nc.tensor.matmul(
                                    acc[:, asl], lhsT,
                                    supp_g[g][:, jj * 129:(jj + 1) * 129],
                                    start=(jj == hg * 4),
                                    stop=(jj == hg * 4 + 3),
                                )
                        W3 = len(icb) * 129
                        dst = osum[:, icb[0] * 129:icb[0] * 129 + W3]
                        if g == 0 and hg == 0:
                            # +1e-30 guards the (measure-zero) all-masked-row
                            # 0/0 case; harmless elsewhere
                            nc.vector.tensor_scalar_add(
                                dst, acc[:, 0:W3], 1e-30)
                        else:
                            nc.vector.tensor_add(dst, dst, acc[:, 0:W3])
                        if g == NGROUPS - 1 and hg == 1:
                            # epilogue inline: normalize + residual + store
                            rc = epp.tile([128, ICB], f32, tag="rc")
                            osr = osum.rearrange("p (i c) -> p i c", c=D + 1)
                            nc.vector.reciprocal(
                                rc[:, 0:len(icb)],
                                osr[:, icb[0]:icb[0] + len(icb), D])
                            for i3, ic in enumerate(icb):
                                of = epp.tile([128, D], f32, tag="of")
                                nc.vector.scalar_tensor_tensor(
                                    of, in0=osum[:, ic * 129:ic * 129 + 128],
                                    scalar=rc[:, i3:i3 + 1],
                                    in1=res[:, ic * 128:(ic + 1) * 128],
                                    op0=mult, op1=add)
                                nc.sync.dma_start(
                                    out=outb[ic * 128:(ic + 1) * 128, :],
                                    in_=of)

    nc.compile()
    return nc


def _get_program():
    main_bf16 = os.environ.get("KERNEL_MAIN_BF16", "1") == "1"
    key = ("prog", main_bf16,
           os.environ.get("KERNEL_GPS_EVERY", "4"),
           os.environ.get("KERNEL_PBUF", "7"),
           os.environ.get("KERNEL_DVE_PRELU", "0"))
    if key not in _cache:
        _cache[key] = _build_program(main_bf16)
    return _cache[key]


def kernel(inputs, adjacency, weight, weight_u, weight_v, bias, proj_w, proj_b):
    from concourse.bass_utils import run_bass_kernel_spmd

    inputs = np.ascontiguousarray(np.asarray(inputs, np.float32))
    adjacency = np.asarray(adjacency, np.float32)
    weight = np.asarray(weight, np.float32)
    weight_u = np.asarray(weight_u, np.float32)
    weight_v = np.asarray(weight_v, np.float32)
    bias = np.asarray(bias, np.float32).reshape(1, H * D)
    proj_w = np.asarray(proj_w, np.float32)
    proj_b = np.asarray(proj_b, np.float32).reshape(H * D)

    nc = _get_program()

    in_maps = []
    for c in range(NCORES):
        h = c // 2
        r0 = (c % 2) * RPC
        hs = slice(h * D, (h + 1) * D)
        # rolled node order: own query rows first
        rolled_inputs = np.roll(inputs, -r0, axis=0)
        inpT_ext = np.ascontiguousarray(rolled_inputs.T)
        adjT_c = np.ascontiguousarray(
            np.roll(adjacency[r0:r0 + RPC, :], -r0, axis=1).T
        ).astype(ml_dtypes.bfloat16)  # exact: adjacency is 0.0/1.0
        in_maps.append({
            "adjT": adjT_c,
            "inpT": inpT_ext,
            "wh": np.ascontiguousarray(weight[:, hs]),
            "whT": np.ascontiguousarray(weight[:, hs].T),
            "uvh": np.ascontiguousarray(
                np.concatenate([weight_u[h], weight_v[h]], axis=1)),
            "br": np.ascontiguousarray(
                np.stack([bias[0, hs], proj_b[hs]], axis=0)),
            "pwh": np.ascontiguousarray(proj_w[:, hs]),
        })

    trace = os.environ.get("KERNEL_TRACE", "0") == "1"
    results = run_bass_kernel_spmd(
        nc, in_maps, core_ids=list(range(NCORES)), trace=trace)
    _cache["last_results"] = results

    out = np.empty((N, H * D), np.float32)
    for c in range(NCORES):
        h = c // 2
        r0 = (c % 2) * RPC
        out[r0:r0 + RPC, h * D:(h + 1) * D] = results.results[c]["outb"]
    return out


# revision 49
# speedup vs baseline: 1.0546x; 1.0035x over previous
"""GAT-style dense-mask attention (gnn_message_passing) on 8 trn2 cores.

Sharding: core c owns head h=c//2 and query rows [r0, r0+2048), r0=(c%2)*2048.
Inputs are pre-sliced/rolled on host so all 8 cores run one identical SPMD
program; outputs are [2048,128] blocks reassembled on host.

Math (per core, node order rolled so own rows come first):
  support = X @ Wh                      [4096, 128]
  f1 = X @ (Wh @ u), f2 = X @ (Wh @ v)  [4096]
  p[j,i] = adj[i,j] * exp(prelu_0.2(f1[j]+f2[i]))     (j on partitions)
  out[i,:] = (p.T @ [support|1])[:, :128] / (p.T @ [support|1])[:, 128]
             + X[i] @ proj_w_h + bias_h + proj_b_h
The mask multiply uses exp underflow (adj=0 -> p=0) instead of the -1e30
trick, which is exactly equivalent because softmax is shift-invariant.

Schedule: stage-2 pools are allocated before stage-1's streaming input pool
so their SBUF ranges are disjoint — attention tiles (ACT prelu/exp, DVE/GPS
mask) overlap the tail of the support pass on PE.
"""

import os

import ml_dtypes
import numpy as np

N = 4096
IN = 512
D = 128
H = 4
NCORES = 8
RPC = N // 2          # query rows per core
JCH = N // 128        # 32 source-node chunks
ICH = RPC // 128      # 16 query-row chunks
JG = 8                # j-chunks per psum-accumulation group
NGROUPS = JCH // JG   # 4
NPAIR = JG // 2       # chunk-pairs per group

_cache = {}


def _build_program(main_bf16: bool):
    import concourse.bacc as bacc
    import concourse.mybir as mybir
    import concourse.tile as tile

    f32 = mybir.dt.float32
    f32r = mybir.dt.float32r
    bf16 = mybir.dt.bfloat16
    mdt = mybir.dt.bfloat16 if main_bf16 else f32
    Prelu = mybir.ActivationFunctionType.Prelu
    Exp = mybir.ActivationFunctionType.Exp
    add = mybir.AluOpType.add
    mult = mybir.AluOpType.mult
    gps_every = int(os.environ.get("KERNEL_GPS_EVERY", "4"))
    pbuf_bufs = int(os.environ.get("KERNEL_PBUF", "7"))

    nc = bacc.Bacc(
        "TRN2",
        target_bir_lowering=False,
        debug=False,
        enable_asserts=False,
        num_devices=NCORES,
    )

    adjT = nc.dram_tensor("adjT", [N, RPC], bf16, kind="ExternalInput").ap()
    inpT = nc.dram_tensor("inpT", [IN, N], f32r, kind="ExternalInput").ap()
    wh = nc.dram_tensor("wh", [IN, D], f32r, kind="ExternalInput").ap()
    whT = nc.dram_tensor("whT", [D, IN], f32r, kind="ExternalInput").ap()
    uvh = nc.dram_tensor("uvh", [D, 2], f32r, kind="ExternalInput").ap()
    br = nc.dram_tensor("br", [2, D], f32, kind="ExternalInput").ap()
    pwh = nc.dram_tensor("pwh", [IN, D], f32r, kind="ExternalInput").ap()
    outb = nc.dram_tensor("outb", [RPC, D], f32, kind="ExternalOutput").ap()

    with tile.TileContext(nc) as tc:
        with tc.tile_pool(name="persist", bufs=1) as persist, \
             tc.tile_pool(name="adjp", bufs=2) as adjp, \
             tc.tile_pool(name="tmpp", bufs=2) as tmpp, \
             tc.tile_pool(name="pbufp", bufs=pbuf_bufs) as pbufp, \
             tc.tile_pool(name="epp", bufs=2) as epp, \
             tc.tile_pool(name="accp", bufs=2, space="PSUM") as accp:
            # supp/f12 are split per j-group so stage-2 consumers only wait
            # on the group's stage-1 writes (Tile deps are tile-granular)
            supp_g = [persist.tile([128, JG * (D + 1)], mdt, tag=f"supp{g}",
                                   name=f"supp{g}") for g in range(NGROUPS)]
            f12_g = [persist.tile([128, JG], f32, tag=f"f12{g}",
                                  name=f"f12{g}") for g in range(NGROUPS)]
            alpha_col = persist.tile([128, 1], f32)          # Prelu slope
            res = persist.tile([128, RPC], f32)              # residual+bias
            bias_bc = persist.tile([128, D], f32)            # (bias+proj_b)
            F2b = persist.tile([128, RPC], f32)              # f2 partition-bcast
            osum = persist.tile([128, ICH * (D + 1)], f32)   # out accumulator
            for g in range(NGROUPS):
                nc.vector.memset(supp_g[g], 1.0)
            nc.vector.memset(alpha_col, 0.2)

            # ---- stage 1: support/f1/residual in one fused PE pass ----
            # rhs columns: [w1 | Wh | proj_w | pad]; all operands are f32r
            # (width 258 >= 256 keeps the PE at 1 cyc/row); f2 is computed
            # separately as rows so F2b is ready early.
            with tc.tile_pool(name="s1c", bufs=1) as s1c, \
                 tc.tile_pool(name="s1p", bufs=2, space="PSUM") as s1p, \
                 tc.tile_pool(name="s1in", bufs=2) as s1in:
                whT_sb = s1c.tile([D, IN], f32r)
                nc.gpsimd.dma_start(out=whT_sb, in_=whT)
                uv_sb = s1c.tile([D, 2], f32r)
                nc.gpsimd.dma_start(out=uv_sb, in_=uvh)
                w12 = s1c.tile([128, 8], f32r)                # w1|w2 per k-chunk
                for kc in range(4):
                    wps = s1p.tile([128, 2], f32, tag="f2ps")
                    nc.tensor.matmul(
                        wps, whT_sb[:, kc * 128:(kc + 1) * 128], uv_sb,
                        start=True, stop=True,
                    )
                    nc.vector.tensor_copy(out=w12[:, 2 * kc:2 * kc + 2], in_=wps)

                rhs_sb = []
                for kc in range(4):
                    t = s1c.tile([128, 258], f32r, tag=f"rhs{kc}")  # col 257 pad
                    nc.vector.memset(t[:, 257:258].bitcast(f32), 0.0)
                    nc.vector.tensor_copy(
                        out=t[:, 0:1], in_=w12[:, 2 * kc:2 * kc + 1])
                    # issued from ACT: it is idle until stage 2 starts, and
                    # gpsimd's queue is busy with whT/uv (w12 critical path)
                    nc.scalar.dma_start(
                        out=t[:, 1:129], in_=wh[kc * 128:(kc + 1) * 128, :])
                    nc.scalar.dma_start(
                        out=t[:, 129:257], in_=pwh[kc * 128:(kc + 1) * 128, :])
                    rhs_sb.append(t)
                # (bias + proj_b) broadcast across partitions
                br2 = s1c.tile([1, 2 * D], f32)
                nc.scalar.dma_start(out=br2[0:1, 0:D], in_=br[0:1, :])
                nc.scalar.dma_start(out=br2[0:1, D:2 * D], in_=br[1:2, :])
                bsum = s1c.tile([1, D], f32)
                nc.vector.tensor_add(bsum, br2[0:1, 0:D], br2[0:1, D:2 * D])
                nc.gpsimd.partition_broadcast(bias_bc, bsum)

                f2row = s1c.tile([1, RPC], f32)

                # Load both own-row input blocks, then ALL f2 matmuls before
                # any support matmul: F2b is the gate for stage-2 activations,
                # so it must be first in the PE stream.
                it_blks = {}
                for blk in range(2):
                    it_blks[blk] = []
                    for kc in range(4):
                        t = s1in.tile([128, 8 * 128], f32r, tag=f"it{kc}")
                        nc.sync.dma_start(
                            out=t,
                            in_=inpT[kc * 128:(kc + 1) * 128,
                                     blk * 1024:(blk + 1) * 1024])
                        it_blks[blk].append(t)
                for blk in range(2):
                    for nchunk in range(2):
                        f2ps = s1p.tile([1, 512], f32, tag="f2ps")
                        for kc in range(4):
                            nc.tensor.matmul(
                                f2ps,
                                w12[:, 2 * kc + 1:2 * kc + 2],
                                it_blks[blk][kc][:, nchunk * 512:
                                                 (nchunk + 1) * 512],
                                start=(kc == 0), stop=(kc == 3),
                            )
                        # on ACT (idle here) so the F2b broadcast isn't
                        # queued behind stage-1's DVE copy stream
                        nc.scalar.copy(
                            out=f2row[0:1, blk * 1024 + nchunk * 512:
                                      blk * 1024 + (nchunk + 1) * 512],
                            in_=f2ps)
                nc.gpsimd.partition_broadcast(F2b, f2row)

                # inputsT streamed in 4 column-blocks of 1024 nodes
                for blk in range(4):
                    if blk < 2:
                        it = it_blks[blk]
                    else:
                        it = []
                        for kc in range(4):
                            t = s1in.tile([128, 8 * 128], f32r, tag=f"it{kc}")
                            nc.sync.dma_start(
                                out=t,
                                in_=inpT[kc * 128:(kc + 1) * 128,
                                         blk * 1024:(blk + 1) * 1024])
                            it.append(t)
                    for jp in range(4):       # pairs of j-chunks
                        jc = blk * 8 + 2 * jp
                        own = jc < ICH
                        # halves bank-aligned: matmul out must stay in a bank
                        ps = s1p.tile([128, 2, 512], f32, tag="ps")
                        for half in range(2):
                            for kc in range(4):
                                lhsT = it[kc][:, (2 * jp + half) * 128:
                                              (2 * jp + half + 1) * 128]
                                nc.tensor.matmul(
                                    ps[:, half, 0:258], lhsT, rhs_sb[kc],
                                    start=(kc == 0), stop=(kc == 3),
                                )
                        jg, jo = jc // JG, jc % JG
                        # strided pair-copies: one DVE op covers both chunks
                        so = supp_g[jg][:, jo * 129:(jo + 2) * 129].rearrange(
                            "p (c w) -> p c w", c=2)[:, :, 0:128]
                        nc.vector.tensor_copy(out=so, in_=ps[:, :, 1:129])
                        nc.vector.tensor_copy(
                            out=f12_g[jg][:, jo:jo + 2], in_=ps[:, :, 0:1])
                        if own:
                            for half in range(2):
                                nc.vector.scalar_tensor_tensor(
                                    res[:, (jc + half) * 128:
                                        (jc + half + 1) * 128],
                                    in0=ps[:, half, 129:257], scalar=0.0,
                                    in1=bias_bc, op0=add, op1=add)

            # ---- stage 2: attention pairs + aggregation ----
            # Pairs of j-chunks share one [128, 2*RPC] tile so exp and the
            # mask multiply run double-width (amortizes fixed op costs).
            n_dve_prelu = int(os.environ.get("KERNEL_DVE_PRELU", "0"))
            dve_prelu = {int((i + 0.5) * (NGROUPS * NPAIR) / n_dve_prelu)
                         for i in range(n_dve_prelu)} if n_dve_prelu else set()
            # row-chunks per psum bank for the output accumulation
            ICB = 3
            ic_blocks = [list(range(s, min(s + ICB, ICH)))
                         for s in range(0, ICH, ICB)]
            for g in range(NGROUPS):
                pair_tiles = []
                for pr in range(NPAIR):
                    idx = g * NPAIR + pr
                    jc0 = g * JG + 2 * pr
                    adj_t = adjp.tile([128, 2 * RPC], bf16, tag="adj")
                    m_t = tmpp.tile([128, 2 * RPC], f32, tag="m")
                    for half in range(2):
                        jc = jc0 + half
                        jo = jc % JG
                        sl = slice(half * RPC, (half + 1) * RPC)
                        nc.sync.dma_start(
                            out=adj_t[:, sl],
                            in_=adjT[jc * 128:(jc + 1) * 128, :])
                        if idx in dve_prelu:
                            # leaky-relu on DVE to offload the ACT wall:
                            # s = f1+f2 (2x ts), then max(s, 0.2s) in place
                            nc.vector.tensor_scalar_add(
                                m_t[:, sl], F2b,
                                f12_g[g][:, jo:jo + 1])
                        else:
                            nc.scalar.activation(
                                m_t[:, sl], F2b, Prelu,
                                bias=f12_g[g][:, jo:jo + 1], scale=1.0,
                                alpha=alpha_col[:, 0:1])
                    if idx in dve_prelu:
                        nc.vector.scalar_tensor_tensor(
                            m_t, in0=m_t, scalar=0.2, in1=m_t,
                            op0=mult, op1=mybir.AluOpType.max)
                    # exp writes bf16 straight into the p tile; the bf16
                    # adjacency mask is applied in place at DVE 2x rate
                    p_t = pbufp.tile([128, 2 * RPC], mdt, tag="pbuf")
                    nc.scalar.activation(p_t, m_t, Exp)
                    eng = nc.gpsimd if idx % gps_every == 0 else nc.vector
                    eng.tensor_mul(p_t, adj_t, p_t)
                    pair_tiles.append(p_t)
                # consume in two half-groups (pairs 0-1, then 2-3) so the
                # matmuls start before the later pairs' masks finish and
                # pbuf slots free earlier; ICB row-chunks share one psum
                # bank so one flush-add covers ICB chunks
                for hg in range(2):
                    for icb in ic_blocks:
                        acc = accp.tile([128, ICB * (D + 1)], f32, tag="acc")
                        for i3, ic in enumerate(icb):
                            asl = slice(i3 * 129, i3 * 129 + 129)
                            for jj in range(hg * 4, hg * 4 + 4):
                                lhsT = pair_tiles[jj // 2][
                                    :, (jj % 2) * RPC + ic * 128:
                                       (jj % 2) * RPC + (ic + 1) * 128]
                                nc.tensor.matmul(
                                    acc[:, asl], lhsT,
                                    supp_g[g][:, jj * 129:(jj + 1) * 129],
                                    start=(jj == hg * 4),
                                    stop=(jj == hg * 4 + 3),
                                )
                        W3 = len(icb) * 129
                        dst = osum[:, icb[0] * 129:icb[0] * 129 + W3]
                        if g == 0 and hg == 0:
                            # +1e-30 guards the (measure-zero) all-masked-row
                            # 0/0 case; harmless elsewhere
                            nc.vector.tensor_scalar_add(
                                dst, acc[:, 0:W3], 1e-30)
                        else:
                            nc.vector.tensor_add(dst, dst, acc[:, 0:W3])
                        if g == NGROUPS - 1 and hg == 1:
                            # epilogue inline: normalize + residual + store
                            rc = epp.tile([128, ICB], f32, tag="rc")
                            osr = osum.rearrange("p (i c) -> p i c", c=D + 1)
                            nc.vector.reciprocal(
                                rc[:, 0:len(icb)],
                                osr[:, icb[0]:icb[0] + len(icb), D])
                            for i3, ic in enumerate(icb):
                                of = epp.tile([128, D], f32, tag="of")
                                nc.vector.scalar_tensor_tensor(
                                    of, in0=osum[:, ic * 129:ic * 129 + 128],
                                    scalar=rc[:, i3:i3 + 1],
                                    in1=res[:, ic * 128:(ic + 1) * 128],
                                    op0=mult, op1=add)
                                nc.sync.dma_start(
                                    out=outb[ic * 128:(ic + 1) * 128, :],
                                    in_=of)

    nc.compile()
    return nc


def _get_program():
    main_bf16 = os.environ.get("KERNEL_MAIN_BF16", "1") == "1"
    key = ("prog", main_bf16,
           os.environ.get("KERNEL_GPS_EVERY", "4"),
           os.environ.get("KERNEL_PBUF", "7"),
           os.environ.get("KERNEL_DVE_PRELU", "0"))
    if key not in _cache:
        _cache[key] = _build_program(main_bf16)
    return _cache[key]


def kernel(inputs, adjacency, weight, weight_u, weight_v, bias, proj_w, proj_b):
    from concourse.bass_utils import run_bass_kernel_spmd

    inputs = np.ascontiguousarray(np.asarray(inputs, np.float32))
    adjacency = np.asarray(adjacency, np.float32)
    weight = np.asarray(weight, np.float32)
    weight_u = np.asarray(weight_u, np.float32)
    weight_v = np.asarray(weight_v, np.float32)
    bias = np.asarray(bias, np.float32).reshape(1, H * D)
    proj_w = np.asarray(proj_w, np.float32)
    proj_b = np.asarray(proj_b, np.float32).reshape(H * D)

    nc = _get_program()

    in_maps = []
    for c in range(NCORES):
        h = c // 2
        r0 = (c % 2) * RPC
        hs = slice(h * D, (h + 1) * D)
        # rolled node order: own query rows first
        rolled_inputs = np.roll(inputs, -r0, axis=0)
        inpT_ext = np.ascontiguousarray(rolled_inputs.T)
        adjT_c = np.ascontiguousarray(
            np.roll(adjacency[r0:r0 + RPC, :], -r0, axis=1).T
        ).astype(ml_dtypes.bfloat16)  # exact: adjacency is 0.0/1.0
        in_maps.append({
            "adjT": adjT_c,
            "inpT": inpT_ext,
            "wh": np.ascontiguousarray(weight[:, hs]),
            "whT": np.ascontiguousarray(weight[:, hs].T),
            "uvh": np.ascontiguousarray(
                np.concatenate([weight_u[h], weight_v[h]], axis=1)),
            "br": np.ascontiguousarray(
                np.stack([bias[0, hs], proj_b[hs]], axis=0)),
            "pwh": np.ascontiguousarray(proj_w[:, hs]),
        })

    trace = os.environ.get("KERNEL_TRACE", "0") == "1"
    results = run_bass_kernel_spmd(
        nc, in_maps, core_ids=list(range(NCORES)), trace=trace)
    _cache["last_results"] = results

    out = np.empty((N, H * D), np.float32)
    for c in range(NCORES):
        h = c // 2
        r0 = (c % 2) * RPC
        out[r0:r0 + RPC, h * D:(h + 1) * D] = results.results[c]["outb"]
    return out
